# revision 3
# baseline (speedup 1.0000x reference)
"""Trainium2 Bass kernel for nn_AlignmentModule.

Data-parallel over batch B=128 across 8 NeuronCores (16 batches/core).
All per-batch attention/MLP compute stays on-chip in bf16 (fp32 PSUM
accumulation); only the tiny final bias add happens on host.

Layout strategy (per batch, per core):
  cT  [D, Lc] = relu(Wa.T @ critT + ba)        (Wa is the natural lhsT)
  eT  [D, Le] = relu(Wa.T @ ehrT  + ba)
  align  [Lc, Le] = cT.T @ eT ;  alignT [Le, Lc] = eT.T @ cT
  exp_cm  [Lc, Le] = exp(align  + (1-cm)*NEG)   (per-partition bias on ACT)
  exp_em  [Le, Lc] = exp(alignT + (1-em)*NEG)
  att_eT [D, Le] = crit.T @ exp_cm ; colsum [1, Le] = ones.T @ exp_cm
  att_cT [D, Lc] = ehr.T  @ exp_em ; rowsum [1, Lc] = ones.T @ exp_em
  normalize via PE ones-outer broadcast + DVE reciprocal/multiply
  r1T [Dout,1] = sum_Lc relu(Wr.T @ [att_cT;critT] + br)   (ACT accum_out)
  r2T [Dout,1] = sum_Le relu(Wr.T @ [att_eT;ehrT] + br)
  final MLP on [4D, 16] columns once per core.

PSUM is time-shared through two uniform rings: p1 = four 1-bank slots
[128,512]f32, p2 = two 2-bank slots [128,2,512]f32 (8 banks total).
"""

import sys

if "/opt/trn_rl_repo" not in sys.path:
    sys.path.insert(0, "/opt/trn_rl_repo")

from contextlib import ExitStack

import ml_dtypes
import numpy as np

B, LC, LE, D, M = 128, 256, 1024, 256, 512
NCORES = 8
NB = B // NCORES  # batches per core
NEG = -1e9

_cache = {}


def _build():
    import concourse.tile as tile
    from concourse import bacc, mybir

    f32 = mybir.dt.float32
    bf16 = mybir.dt.bfloat16
    Relu = mybir.ActivationFunctionType.Relu
    Exp = mybir.ActivationFunctionType.Exp
    mult = mybir.AluOpType.mult

    nc = bacc.Bacc("TRN2", target_bir_lowering=False, debug=False,
                   num_devices=NCORES)

    def din(name, shape, dt):
        return nc.dram_tensor(name, shape, dt, kind="ExternalInput").ap()

    crit = din("criteria", [NB, LC, D], bf16)
    critT = din("criteriaT", [NB, D, LC], bf16)
    ehr = din("ehr", [NB, LE, D], bf16)
    ehrT = din("ehrT", [NB, D, LE], bf16)
    ncm = din("ncm", [128, NB * 2], f32)
    nem = din("nem", [128, NB * 8], f32)
    wa_d = din("Wa", [D, D], bf16)
    wr_d = din("Wr", [2 * D, D], bf16)
    wm_d = din("Wm", [4 * D, M], bf16)
    wo_d = din("Wo", [M, 3], bf16)
    ba_d = din("ba", [128, 2], f32)
    br_d = din("br", [128, 2], f32)
    bm_d = din("bm", [128, 4], f32)
    out_d = nc.dram_tensor("out", [NB, 3], f32, kind="ExternalOutput").ap()

    with tile.TileContext(nc) as tc, ExitStack() as ctx:
        const = ctx.enter_context(tc.tile_pool(name="const", bufs=1))
        sb = ctx.enter_context(tc.tile_pool(name="sb", bufs=2))
        scr = ctx.enter_context(tc.tile_pool(name="scr", bufs=4))
        ps1 = ctx.enter_context(tc.tile_pool(name="ps1", bufs=4, space="PSUM"))
        ps2 = ctx.enter_context(tc.tile_pool(name="ps2", bufs=2, space="PSUM"))

        def p1():
            return ps1.tile([128, 512], f32, tag="p1", name="p1")

        def p2():
            return ps2.tile([128, 2, 512], f32, tag="p2", name="p2")

        # ---- constants (loaded once) ----
        wa = const.tile([128, 2, D], bf16)
        for k in range(2):
            nc.sync.dma_start(wa[:, k, :], wa_d[k * 128:(k + 1) * 128, :])
        wr = const.tile([128, 4, D], bf16)
        for k in range(4):
            nc.sync.dma_start(wr[:, k, :], wr_d[k * 128:(k + 1) * 128, :])
        wm = const.tile([128, 8, M], bf16)
        for k in range(8):
            nc.sync.dma_start(wm[:, k, :], wm_d[k * 128:(k + 1) * 128, :])
        wo = const.tile([128, 4, 3], bf16)
        for k in range(4):
            nc.sync.dma_start(wo[:, k, :], wo_d[k * 128:(k + 1) * 128, :])
        ba = const.tile([128, 2], f32)
        nc.sync.dma_start(ba[:, :], ba_d[:, :])
        br = const.tile([128, 2], f32)
        nc.sync.dma_start(br[:, :], br_d[:, :])
        bm = const.tile([128, 4], f32)
        nc.sync.dma_start(bm[:, :], bm_d[:, :])
        ncm_sb = const.tile([128, NB * 2], f32)
        nc.sync.dma_start(ncm_sb[:, :], ncm[:, :])
        nem_sb = const.tile([128, NB * 8], f32)
        nc.sync.dma_start(nem_sb[:, :], nem[:, :])
        ones_col = const.tile([128, 1], bf16)
        nc.vector.memset(ones_col[:, :], 1.0)
        ones_row = const.tile([1, 128], f32)
        nc.vector.memset(ones_row[:, :], 1.0)
        r1t = const.tile([128, 2, NB], f32)
        r2t = const.tile([128, 2, NB], f32)

        for b in range(NB):
            # ---- loads ----
            cr_n = sb.tile([128, 2, D], bf16, tag="cr_n")
            cr_t = sb.tile([128, 2, LC], bf16, tag="cr_t")
            for j in range(2):
                nc.sync.dma_start(cr_n[:, j, :], crit[b, j * 128:(j + 1) * 128, :])
                nc.sync.dma_start(cr_t[:, j, :], critT[b, j * 128:(j + 1) * 128, :])
            eh_n = sb.tile([128, 8, D], bf16, tag="eh_n")
            for j in range(8):
                nc.sync.dma_start(eh_n[:, j, :], ehr[b, j * 128:(j + 1) * 128, :])
            eh_t = sb.tile([128, 2, LE], bf16, tag="eh_t")
            for j in range(2):
                nc.sync.dma_start(eh_t[:, j, :], ehrT[b, j * 128:(j + 1) * 128, :])

            # ---- cT = relu(Wa.T @ critT + ba), eT likewise ----
            ct = sb.tile([128, 2, LC], bf16, tag="ct")
            for m in range(2):
                t = p1()
                for k in range(2):
                    nc.tensor.matmul(t[:, 0:LC], wa[:, k, m * 128:(m + 1) * 128],
                                     cr_t[:, k, :], start=(k == 0), stop=(k == 1))
                nc.scalar.activation(ct[:, m, :], t[:, 0:LC], Relu,
                                     bias=ba[:, m:m + 1])
            et = sb.tile([128, 2, LE], bf16, tag="et")
            for m in range(2):
                t = p2()
                for n in range(2):
                    for k in range(2):
                        nc.tensor.matmul(t[:, n, :],
                                         wa[:, k, m * 128:(m + 1) * 128],
                                         eh_t[:, k, n * 512:(n + 1) * 512],
                                         start=(k == 0), stop=(k == 1))
                for n in range(2):
                    nc.scalar.activation(et[:, m, n * 512:(n + 1) * 512],
                                         t[:, n, :], Relu, bias=ba[:, m:m + 1])

            # ---- align = cT.T @ eT -> exp(+cm mask); alignT -> exp(+em) ----
            exal = sb.tile([128, 2, LE], bf16, tag="exal")
            for m in range(2):
                t = p2()
                for n in range(2):
                    for k in range(2):
                        nc.tensor.matmul(t[:, n, :],
                                         ct[:, k, m * 128:(m + 1) * 128],
                                         et[:, k, n * 512:(n + 1) * 512],
                                         start=(k == 0), stop=(k == 1))
                for n in range(2):
                    nc.scalar.activation(exal[:, m, n * 512:(n + 1) * 512],
                                         t[:, n, :], Exp,
                                         bias=ncm_sb[:, 2 * b + m:2 * b + m + 1])
            exalt = sb.tile([128, 8, LC], bf16, tag="exalt")
            for m8 in range(8):
                t = p1()
                for k in range(2):
                    nc.tensor.matmul(t[:, 0:LC],
                                     et[:, k, m8 * 128:(m8 + 1) * 128],
                                     ct[:, k, :], start=(k == 0), stop=(k == 1))
                nc.scalar.activation(exalt[:, m8, :], t[:, 0:LC], Exp,
                                     bias=nem_sb[:, 8 * b + m8:8 * b + m8 + 1])

            # ---- att_eT = crit.T @ exp_cm ; colsum ; normalize ----
            rc = sb.tile([128, 2, 512], f32, tag="rc")
            for n in range(2):
                t = p1()
                for k in range(2):
                    nc.tensor.matmul(t[0:1, :], ones_col[:, 0:1],
                                     exal[:, k, n * 512:(n + 1) * 512],
                                     start=(k == 0), stop=(k == 1))
                c_sb = scr.tile([1, 512], f32, tag="cs_sb")
                nc.scalar.copy(c_sb[0:1, :], t[0:1, :])
                t2 = p1()
                nc.tensor.matmul(t2[:, :], ones_row[0:1, :], c_sb[0:1, :],
                                 start=True, stop=True)
                nc.vector.reciprocal_approx_fast(rc[:, n, :], t2[:, :])
            aen = sb.tile([128, 2, LE], bf16, tag="aen")
            for m in range(2):
                t = p2()
                for n in range(2):
                    for k in range(2):
                        nc.tensor.matmul(t[:, n, :],
                                         cr_n[:, k, m * 128:(m + 1) * 128],
                                         exal[:, k, n * 512:(n + 1) * 512],
                                         start=(k == 0), stop=(k == 1))
                for n in range(2):
                    nc.vector.tensor_tensor(aen[:, m, n * 512:(n + 1) * 512],
                                            t[:, n, :], rc[:, n, :], op=mult)

            # ---- att_cT = ehr.T @ exp_em ; rowsum ; normalize ----
            rrc = sb.tile([128, LC], f32, tag="rrc")
            t = p1()
            for k8 in range(8):
                nc.tensor.matmul(t[0:1, 0:LC], ones_col[:, 0:1],
                                 exalt[:, k8, :], start=(k8 == 0),
                                 stop=(k8 == 7))
            r_sb = scr.tile([1, 512], f32, tag="cs_sb")
            nc.scalar.copy(r_sb[0:1, 0:LC], t[0:1, 0:LC])
            t2 = p1()
            nc.tensor.matmul(t2[:, 0:LC], ones_row[0:1, :], r_sb[0:1, 0:LC],
                             start=True, stop=True)
            nc.vector.reciprocal_approx_fast(rrc[:, :], t2[:, 0:LC])
            acn = sb.tile([128, 2, LC], bf16, tag="acn")
            for m in range(2):
                t = p1()
                for k8 in range(8):
                    nc.tensor.matmul(t[:, 0:LC],
                                     eh_n[:, k8, m * 128:(m + 1) * 128],
                                     exalt[:, k8, :], start=(k8 == 0),
                                     stop=(k8 == 7))
                nc.vector.tensor_tensor(acn[:, m, :], t[:, 0:LC], rrc[:, :],
                                        op=mult)

            # ---- r1 = sum_Lc relu(Wr.T @ [att_cT; critT] + br) ----
            rhs1 = [acn[:, 0, :], acn[:, 1, :], cr_t[:, 0, :], cr_t[:, 1, :]]
            for m in range(2):
                t = p1()
                for k4 in range(4):
                    nc.tensor.matmul(t[:, 0:LC],
                                     wr[:, k4, m * 128:(m + 1) * 128],
                                     rhs1[k4], start=(k4 == 0), stop=(k4 == 3))
                s = scr.tile([128, 2, 512], bf16, tag="scr")
                nc.scalar.activation(s[:, 0, 0:LC], t[:, 0:LC], Relu,
                                     bias=br[:, m:m + 1],
                                     accum_out=r1t[:, m, b:b + 1])

            # ---- r2 = sum_Le relu(Wr.T @ [att_eT; ehrT] + br) ----
            for m in range(2):
                t = p2()
                for n in range(2):
                    for k4 in range(4):
                        src = aen if k4 < 2 else eh_t
                        kk = k4 % 2
                        nc.tensor.matmul(t[:, n, :],
                                         wr[:, k4, m * 128:(m + 1) * 128],
                                         src[:, kk, n * 512:(n + 1) * 512],
                                         start=(k4 == 0), stop=(k4 == 3))
                s = scr.tile([128, 2, 512], bf16, tag="scr")
                nc.scalar.activation(s[:, :, :], t[:, :, :], Relu,
                                     bias=br[:, m:m + 1],
                                     accum_out=r2t[:, m, b:b + 1])

        # ---- final MLP once per core ----
        mt = const.tile([128, 8, NB], bf16)
        for m in range(2):
            nc.vector.tensor_copy(mt[:, m, :], r1t[:, m, :])
            nc.vector.tensor_copy(mt[:, 2 + m, :], r2t[:, m, :])
            nc.vector.tensor_tensor(mt[:, 4 + m, :], r1t[:, m, :], r2t[:, m, :],
                                    op=mult)
            nc.vector.tensor_sub(mt[:, 6 + m, :], r1t[:, m, :], r2t[:, m, :])
        ht = const.tile([128, 4, NB], bf16)
        for m4 in range(4):
            t = p1()
            for k8 in range(8):
                nc.tensor.matmul(t[:, 0:NB],
                                 wm[:, k8, m4 * 128:(m4 + 1) * 128],
                                 mt[:, k8, :], start=(k8 == 0), stop=(k8 == 7))
            nc.scalar.activation(ht[:, m4, :], t[:, 0:NB], Relu,
                                 bias=bm[:, m4:m4 + 1])
        o_sb = const.tile([NB, 3], f32)
        t = p1()
        for k4 in range(4):
            nc.tensor.matmul(t[0:NB, 0:3], ht[:, k4, :], wo[:, k4, :],
                             start=(k4 == 0), stop=(k4 == 3))
        nc.vector.tensor_copy(o_sb[:, :], t[0:NB, 0:3])
        nc.sync.dma_start(out_d[:, :], o_sb[:, :])

    nc.compile()
    return nc


def _prep_inputs(inputs):
    """Host-side shard + layout prep. Returns (in_maps, bo)."""
    bf16 = ml_dtypes.bfloat16
    crit = np.asarray(inputs["criteria"], np.float32)
    ehr = np.asarray(inputs["ehr"], np.float32)
    cm = np.asarray(inputs["criteria_mask"], np.float32)
    em = np.asarray(inputs["ehr_mask"], np.float32)
    ncm = ((1.0 - cm) * NEG).astype(np.float32)  # [B, LC]
    nem = ((1.0 - em) * NEG).astype(np.float32)  # [B, LE]

    critb = crit.astype(bf16)
    critTb = np.ascontiguousarray(crit.transpose(0, 2, 1)).astype(bf16)
    ehrb = ehr.astype(bf16)
    ehrTb = np.ascontiguousarray(ehr.transpose(0, 2, 1)).astype(bf16)

    wa = np.asarray(inputs["Wa"], np.float32).astype(bf16)
    wr = np.asarray(inputs["Wr"], np.float32).astype(bf16)
    wm = np.asarray(inputs["Wm"], np.float32).astype(bf16)
    wo = np.asarray(inputs["Wo"], np.float32).astype(bf16)
    ba = np.asarray(inputs["ba"], np.float32).reshape(2, 128).T.copy()
    br = np.asarray(inputs["br"], np.float32).reshape(2, 128).T.copy()
    bm = np.asarray(inputs["bm"], np.float32).reshape(4, 128).T.copy()
    bo = np.asarray(inputs["bo"], np.float32)

    in_maps = []
    for c in range(NCORES):
        lo, hi = c * NB, (c + 1) * NB
        ncm_c = ncm[lo:hi].reshape(NB, 2, 128).transpose(2, 0, 1)
        nem_c = nem[lo:hi].reshape(NB, 8, 128).transpose(2, 0, 1)
        in_maps.append({
            "criteria": critb[lo:hi],
            "criteriaT": critTb[lo:hi],
            "ehr": ehrb[lo:hi],
            "ehrT": ehrTb[lo:hi],
            "ncm": np.ascontiguousarray(ncm_c.reshape(128, NB * 2)),
            "nem": np.ascontiguousarray(nem_c.reshape(128, NB * 8)),
            "Wa": wa, "Wr": wr, "Wm": wm, "Wo": wo,
            "ba": ba, "br": br, "bm": bm,
        })
    return in_maps, bo


def _run(inputs, trace=False):
    from concourse.bass_utils import run_bass_kernel_spmd

    if "nc" not in _cache:
        _cache["nc"] = _build()
    nc = _cache["nc"]
    in_maps, bo = _prep_inputs(inputs)
    res = run_bass_kernel_spmd(nc, in_maps, core_ids=list(range(NCORES)),
                               trace=trace)
    out = np.concatenate([np.asarray(res.results[c]["out"], np.float32)
                          for c in range(NCORES)], axis=0)
    out = out + bo[None, :]
    return out, res


def kernel(**inputs):
    out, _ = _run(inputs, trace=False)
    return out


# revision 5
# speedup vs baseline: 1.1936x; 1.1936x over previous
"""Trainium2 Bass kernel for nn_AlignmentModule.

Data-parallel over batch B=128 across 8 NeuronCores (16 batches/core).
All per-batch attention/MLP compute stays on-chip in bf16 (fp32 PSUM
accumulation); only the tiny final bias add happens on host.

Layout strategy (per batch, per core):
  cT  [D, Lc] = relu(Wa.T @ critT + ba)        (Wa is the natural lhsT)
  eT  [D, Le] = relu(Wa.T @ ehrT  + ba)
  align  [Lc, Le] = cT.T @ eT ;  alignT [Le, Lc] = eT.T @ cT
  exp_cm  [Lc, Le] = exp(align  + (1-cm)*NEG)   (per-partition bias on ACT)
  exp_em  [Le, Lc] = exp(alignT + (1-em)*NEG)
  att_eT [D, Le] = crit.T @ exp_cm ; colsum [1, Le] = ones.T @ exp_cm
  att_cT [D, Lc] = ehr.T  @ exp_em ; rowsum [1, Lc] = ones.T @ exp_em
  normalize via PE ones-outer broadcast + DVE reciprocal/multiply
  r1T [Dout,1] = sum_Lc relu(Wr.T @ [att_cT;critT] + br)   (DVE accum_out)
  r2T [Dout,1] = sum_Le relu(Wr.T @ [att_eT;ehrT] + br)    (ACT accum_out)
  final MLP on [4D, 16] columns once per core.

PSUM is a uniform ring of eight 1-bank [128,512]f32 slots. Batches are
emitted pairwise-interleaved so the PE stream always has independent
matmul groups to hide ACT/DVE handoff latency (keeps HAM at 2.4 GHz).
Engine balance: exp/r2-reduce on ScalarE; relu-bias copies, softmax
reciprocals, normalize-multiplies and r1-reduce on VectorE.
"""

import sys

if "/opt/trn_rl_repo" not in sys.path:
    sys.path.insert(0, "/opt/trn_rl_repo")

from contextlib import ExitStack

import ml_dtypes
import numpy as np

B, LC, LE, D, M = 128, 256, 1024, 256, 512
NCORES = 8
NB = B // NCORES  # batches per core
NEG = -1e9

_cache = {}


def _build():
    import concourse.tile as tile
    from concourse import bacc, mybir

    f32 = mybir.dt.float32
    bf16 = mybir.dt.bfloat16
    Relu = mybir.ActivationFunctionType.Relu
    Exp = mybir.ActivationFunctionType.Exp
    mult = mybir.AluOpType.mult
    add = mybir.AluOpType.add
    amax = mybir.AluOpType.max

    nc = bacc.Bacc("TRN2", target_bir_lowering=False, debug=False,
                   num_devices=NCORES)

    def din(name, shape, dt):
        return nc.dram_tensor(name, shape, dt, kind="ExternalInput").ap()

    crit = din("criteria", [NB, LC, D], bf16)
    critT = din("criteriaT", [NB, D, LC], bf16)
    ehr = din("ehr", [NB, LE, D], bf16)
    ehrT = din("ehrT", [NB, D, LE], bf16)
    ncm = din("ncm", [128, NB * 2], f32)
    nem = din("nem", [128, NB * 8], f32)
    wa_d = din("Wa", [D, D], bf16)
    wr_d = din("Wr", [2 * D, D], bf16)
    wm_d = din("Wm", [4 * D, M], bf16)
    wo_d = din("Wo", [M, 3], bf16)
    ba_d = din("ba", [128, 2], f32)
    br_d = din("br", [128, 2], f32)
    bm_d = din("bm", [128, 4], f32)
    out_d = nc.dram_tensor("out", [NB, 3], f32, kind="ExternalOutput").ap()

    with tile.TileContext(nc) as tc, ExitStack() as ctx:
        const = ctx.enter_context(tc.tile_pool(name="const", bufs=1))
        sb = ctx.enter_context(tc.tile_pool(name="sb", bufs=3))
        scr = ctx.enter_context(tc.tile_pool(name="scr", bufs=6))
        ps = ctx.enter_context(tc.tile_pool(name="ps", bufs=8, space="PSUM"))

        def pt():
            return ps.tile([128, 512], f32, tag="p", name="p")

        # ---- constants (loaded once) ----
        wa = const.tile([128, 2, D], bf16)
        for k in range(2):
            nc.sync.dma_start(wa[:, k, :], wa_d[k * 128:(k + 1) * 128, :])
        wr = const.tile([128, 4, D], bf16)
        for k in range(4):
            nc.sync.dma_start(wr[:, k, :], wr_d[k * 128:(k + 1) * 128, :])
        wm = const.tile([128, 8, M], bf16)
        for k in range(8):
            nc.sync.dma_start(wm[:, k, :], wm_d[k * 128:(k + 1) * 128, :])
        wo = const.tile([128, 4, 3], bf16)
        for k in range(4):
            nc.sync.dma_start(wo[:, k, :], wo_d[k * 128:(k + 1) * 128, :])
        ba = const.tile([128, 2], f32)
        nc.sync.dma_start(ba[:, :], ba_d[:, :])
        br = const.tile([128, 2], f32)
        nc.sync.dma_start(br[:, :], br_d[:, :])
        bm = const.tile([128, 4], f32)
        nc.sync.dma_start(bm[:, :], bm_d[:, :])
        ncm_sb = const.tile([128, NB * 2], f32)
        nc.sync.dma_start(ncm_sb[:, :], ncm[:, :])
        nem_sb = const.tile([128, NB * 8], f32)
        nc.sync.dma_start(nem_sb[:, :], nem[:, :])
        ones_col = const.tile([128, 1], bf16)
        nc.vector.memset(ones_col[:, :], 1.0)
        ones_row = const.tile([1, 128], f32)
        nc.vector.memset(ones_row[:, :], 1.0)
        zeros = const.tile([128, 512], bf16)
        nc.vector.memset(zeros[:, :], 0.0)
        r1t = const.tile([128, 2, NB], f32)
        r2t4 = const.tile([128, 2, 2, NB], f32)  # (dout-chunk, n-half, batch)

        S = {}  # per-batch live tiles

        def st_load(b):
            s = S[b] = {}
            s["cr_n"] = sb.tile([128, 2, D], bf16, tag="cr_n", name="cr_n")
            s["cr_t"] = sb.tile([128, 2, LC], bf16, tag="cr_t", name="cr_t")
            for j in range(2):
                nc.sync.dma_start(s["cr_n"][:, j, :],
                                  crit[b, j * 128:(j + 1) * 128, :])
                nc.sync.dma_start(s["cr_t"][:, j, :],
                                  critT[b, j * 128:(j + 1) * 128, :])
            s["eh_n"] = sb.tile([128, 8, D], bf16, tag="eh_n", name="eh_n")
            for j in range(8):
                nc.sync.dma_start(s["eh_n"][:, j, :],
                                  ehr[b, j * 128:(j + 1) * 128, :])
            s["eh_t"] = sb.tile([128, 2, LE], bf16, tag="eh_t", name="eh_t")
            for j in range(2):
                nc.sync.dma_start(s["eh_t"][:, j, :],
                                  ehrT[b, j * 128:(j + 1) * 128, :])

        def st_ct_et(b):
            s = S[b]
            s["ct"] = sb.tile([128, 2, LC], bf16, tag="ct", name="ct")
            for m in range(2):
                t = pt()
                for k in range(2):
                    nc.tensor.matmul(t[:, 0:LC],
                                     wa[:, k, m * 128:(m + 1) * 128],
                                     s["cr_t"][:, k, :],
                                     start=(k == 0), stop=(k == 1))
                nc.vector.scalar_tensor_tensor(s["ct"][:, m, :], t[:, 0:LC],
                                               ba[:, m:m + 1], zeros[:, 0:LC],
                                               add, amax)
            s["et"] = sb.tile([128, 2, LE], bf16, tag="et", name="et")
            for m in range(2):
                for n in range(2):
                    t = pt()
                    for k in range(2):
                        nc.tensor.matmul(t[:, :],
                                         wa[:, k, m * 128:(m + 1) * 128],
                                         s["eh_t"][:, k, n * 512:(n + 1) * 512],
                                         start=(k == 0), stop=(k == 1))
                    nc.vector.scalar_tensor_tensor(
                        s["et"][:, m, n * 512:(n + 1) * 512], t[:, :],
                        ba[:, m:m + 1], zeros[:, :], add, amax)

        def st_align(b):
            s = S[b]
            s["exal"] = sb.tile([128, 2, LE], bf16, tag="exal", name="exal")
            for m in range(2):
                for n in range(2):
                    t = pt()
                    for k in range(2):
                        nc.tensor.matmul(t[:, :],
                                         s["ct"][:, k, m * 128:(m + 1) * 128],
                                         s["et"][:, k, n * 512:(n + 1) * 512],
                                         start=(k == 0), stop=(k == 1))
                    nc.scalar.activation(
                        s["exal"][:, m, n * 512:(n + 1) * 512], t[:, :], Exp,
                        bias=ncm_sb[:, 2 * b + m:2 * b + m + 1])

        def st_alignT(b):
            s = S[b]
            s["exalt"] = sb.tile([128, 8, LC], bf16, tag="exalt", name="exalt")
            for m8 in range(8):
                t = pt()
                for k in range(2):
                    nc.tensor.matmul(t[:, 0:LC],
                                     s["et"][:, k, m8 * 128:(m8 + 1) * 128],
                                     s["ct"][:, k, :],
                                     start=(k == 0), stop=(k == 1))
                nc.scalar.activation(s["exalt"][:, m8, :], t[:, 0:LC], Exp,
                                     bias=nem_sb[:, 8 * b + m8:8 * b + m8 + 1])

        def st_ae(b):
            s = S[b]
            # colsum -> reciprocal broadcast
            s["rc"] = sb.tile([128, 2, 512], f32, tag="rc", name="rc")
            for n in range(2):
                t = pt()
                for k in range(2):
                    nc.tensor.matmul(t[0:1, :], ones_col[:, 0:1],
                                     s["exal"][:, k, n * 512:(n + 1) * 512],
                                     start=(k == 0), stop=(k == 1))
                c_sb = scr.tile([1, 512], f32, tag="cs_sb", name="cs_sb")
                nc.scalar.copy(c_sb[0:1, :], t[0:1, :])
                t2 = pt()
                nc.tensor.matmul(t2[:, :], ones_row[0:1, :], c_sb[0:1, :],
                                 start=True, stop=True)
                nc.vector.reciprocal_approx_fast(s["rc"][:, n, :], t2[:, :])
            # att_eT, normalized
            s["aen"] = sb.tile([128, 2, LE], bf16, tag="aen", name="aen")
            for m in range(2):
                for n in range(2):
                    t = pt()
                    for k in range(2):
                        nc.tensor.matmul(t[:, :],
                                         s["cr_n"][:, k, m * 128:(m + 1) * 128],
                                         s["exal"][:, k, n * 512:(n + 1) * 512],
                                         start=(k == 0), stop=(k == 1))
                    nc.vector.tensor_tensor(
                        s["aen"][:, m, n * 512:(n + 1) * 512], t[:, :],
                        s["rc"][:, n, :], op=mult)

        def st_ac(b):
            s = S[b]
            # rowsum -> reciprocal broadcast
            s["rrc"] = sb.tile([128, LC], f32, tag="rrc", name="rrc")
            t = pt()
            for k8 in range(8):
                nc.tensor.matmul(t[0:1, 0:LC], ones_col[:, 0:1],
                                 s["exalt"][:, k8, :],
                                 start=(k8 == 0), stop=(k8 == 7))
            r_sb = scr.tile([1, 512], f32, tag="cs_sb", name="cs_sb")
            nc.scalar.copy(r_sb[0:1, 0:LC], t[0:1, 0:LC])
            t2 = pt()
            nc.tensor.matmul(t2[:, 0:LC], ones_row[0:1, :], r_sb[0:1, 0:LC],
                             start=True, stop=True)
            nc.vector.reciprocal_approx_fast(s["rrc"][:, :], t2[:, 0:LC])
            # att_cT, normalized
            s["acn"] = sb.tile([128, 2, LC], bf16, tag="acn", name="acn")
            for m in range(2):
                t = pt()
                for k8 in range(8):
                    nc.tensor.matmul(t[:, 0:LC],
                                     s["eh_n"][:, k8, m * 128:(m + 1) * 128],
                                     s["exalt"][:, k8, :],
                                     start=(k8 == 0), stop=(k8 == 7))
                nc.vector.tensor_tensor(s["acn"][:, m, :], t[:, 0:LC],
                                        s["rrc"][:, :], op=mult)

        def st_r1r2(b):
            s = S[b]
            rhs1 = [s["acn"][:, 0, :], s["acn"][:, 1, :],
                    s["cr_t"][:, 0, :], s["cr_t"][:, 1, :]]
            for m in range(2):
                t = pt()
                for k4 in range(4):
                    nc.tensor.matmul(t[:, 0:LC],
                                     wr[:, k4, m * 128:(m + 1) * 128],
                                     rhs1[k4], start=(k4 == 0), stop=(k4 == 3))
                o = scr.tile([128, 512], bf16, tag="scr", name="scr")
                nc.vector.scalar_tensor_tensor(o[:, 0:LC], t[:, 0:LC],
                                               br[:, m:m + 1], zeros[:, 0:LC],
                                               add, amax,
                                               accum_out=r1t[:, m, b:b + 1])
                for n in range(2):
                    t = pt()
                    for k4 in range(4):
                        src = s["aen"] if k4 < 2 else s["eh_t"]
                        kk = k4 % 2
                        nc.tensor.matmul(t[:, :],
                                         wr[:, k4, m * 128:(m + 1) * 128],
                                         src[:, kk, n * 512:(n + 1) * 512],
                                         start=(k4 == 0), stop=(k4 == 3))
                    o = scr.tile([128, 512], bf16, tag="scr", name="scr")
                    nc.scalar.activation(o[:, :], t[:, :], Relu,
                                         bias=br[:, m:m + 1],
                                         accum_out=r2t4[:, m, n, b:b + 1])
            S[b] = None  # release references

        stages = [st_ct_et, st_align, st_alignT, st_ae, st_ac, st_r1r2]
        PAIR = 2
        for bp in range(0, NB, PAIR):
            pair = range(bp, min(bp + PAIR, NB))
            for b in pair:
                st_load(b)
            for st in stages:
                for b in pair:
                    st(b)

        # ---- final MLP once per core ----
        r2s = const.tile([128, 2, NB], f32)
        for m in range(2):
            nc.vector.tensor_add(r2s[:, m, :], r2t4[:, m, 0, :],
                                 r2t4[:, m, 1, :])
        mt = const.tile([128, 8, NB], bf16)
        for m in range(2):
            nc.vector.tensor_copy(mt[:, m, :], r1t[:, m, :])
            nc.vector.tensor_copy(mt[:, 2 + m, :], r2s[:, m, :])
            nc.vector.tensor_tensor(mt[:, 4 + m, :], r1t[:, m, :],
                                    r2s[:, m, :], op=mult)
            nc.vector.tensor_sub(mt[:, 6 + m, :], r1t[:, m, :], r2s[:, m, :])
        ht = const.tile([128, 4, NB], bf16)
        for m4 in range(4):
            t = pt()
            for k8 in range(8):
                nc.tensor.matmul(t[:, 0:NB],
                                 wm[:, k8, m4 * 128:(m4 + 1) * 128],
                                 mt[:, k8, :], start=(k8 == 0), stop=(k8 == 7))
            nc.scalar.activation(ht[:, m4, :], t[:, 0:NB], Relu,
                                 bias=bm[:, m4:m4 + 1])
        o_sb = const.tile([NB, 3], f32)
        t = pt()
        for k4 in range(4):
            nc.tensor.matmul(t[0:NB, 0:3], ht[:, k4, :], wo[:, k4, :],
                             start=(k4 == 0), stop=(k4 == 3))
        nc.vector.tensor_copy(o_sb[:, :], t[0:NB, 0:3])
        nc.sync.dma_start(out_d[:, :], o_sb[:, :])

    nc.compile()
    return nc


def _prep_inputs(inputs):
    """Host-side shard + layout prep. Returns (in_maps, bo)."""
    bf16 = ml_dtypes.bfloat16
    crit = np.asarray(inputs["criteria"], np.float32)
    ehr = np.asarray(inputs["ehr"], np.float32)
    cm = np.asarray(inputs["criteria_mask"], np.float32)
    em = np.asarray(inputs["ehr_mask"], np.float32)
    ncm = ((1.0 - cm) * NEG).astype(np.float32)  # [B, LC]
    nem = ((1.0 - em) * NEG).astype(np.float32)  # [B, LE]

    critb = crit.astype(bf16)
    critTb = np.ascontiguousarray(crit.transpose(0, 2, 1)).astype(bf16)
    ehrb = ehr.astype(bf16)
    ehrTb = np.ascontiguousarray(ehr.transpose(0, 2, 1)).astype(bf16)

    wa = np.asarray(inputs["Wa"], np.float32).astype(bf16)
    wr = np.asarray(inputs["Wr"], np.float32).astype(bf16)
    wm = np.asarray(inputs["Wm"], np.float32).astype(bf16)
    wo = np.asarray(inputs["Wo"], np.float32).astype(bf16)
    ba = np.asarray(inputs["ba"], np.float32).reshape(2, 128).T.copy()
    br = np.asarray(inputs["br"], np.float32).reshape(2, 128).T.copy()
    bm = np.asarray(inputs["bm"], np.float32).reshape(4, 128).T.copy()
    bo = np.asarray(inputs["bo"], np.float32)

    in_maps = []
    for c in range(NCORES):
        lo, hi = c * NB, (c + 1) * NB
        ncm_c = ncm[lo:hi].reshape(NB, 2, 128).transpose(2, 0, 1)
        nem_c = nem[lo:hi].reshape(NB, 8, 128).transpose(2, 0, 1)
        in_maps.append({
            "criteria": critb[lo:hi],
            "criteriaT": critTb[lo:hi],
            "ehr": ehrb[lo:hi],
            "ehrT": ehrTb[lo:hi],
            "ncm": np.ascontiguousarray(ncm_c.reshape(128, NB * 2)),
            "nem": np.ascontiguousarray(nem_c.reshape(128, NB * 8)),
            "Wa": wa, "Wr": wr, "Wm": wm, "Wo": wo,
            "ba": ba, "br": br, "bm": bm,
        })
    return in_maps, bo


def _run(inputs, trace=False):
    from concourse.bass_utils import run_bass_kernel_spmd

    if "nc" not in _cache:
        _cache["nc"] = _build()
    nc = _cache["nc"]
    in_maps, bo = _prep_inputs(inputs)
    res = run_bass_kernel_spmd(nc, in_maps, core_ids=list(range(NCORES)),
                               trace=trace)
    out = np.concatenate([np.asarray(res.results[c]["out"], np.float32)
                          for c in range(NCORES)], axis=0)
    out = out + bo[None, :]
    return out, res


def kernel(**inputs):
    out, _ = _run(inputs, trace=False)
    return out
